# revision 23
# baseline (speedup 1.0000x reference)
"""Trainium2 Bass kernel for nn_MeanAligning (VQ codebook mean-aligning loss).

Math (see reference):
    count[k] = sum_nhw code[nhw, k]
    num[k,c] = sum_nhw code[nhw, k] * quantized[nhw, c]
    mean     = num / count (count==0 -> mean 0)
    loss     = sum_{k: count>0} ||codebook[k] - mean[k]||^2 / (n_valid * C)

Sharding: K-parallel over the 4096 codebook entries — each of the 8 cores
gets a contiguous 512-column slice of `code` and ALL positions, so each
core owns the *complete* count/num for its K-shard. Only a tiny [1,4]
partial crosses cores at the end (summed on host as the unshard step).

Device pipeline per core:
  - `code` is staged host-side as fp8e4 (one-hot 0/1 values are exact in
    fp8e4 — a lossless relayout, 4x less HBM traffic than f32) and
    `quant|ones` as fp8e4 in j-major blocks so the DoubleRow pair stride
    (64*33 = 2112) is 16-aligned with no padding.
  - PSUM-accumulated DoubleRow matmuls: lhsT = [quant|ones] [128, 2, 33],
    rhs = code [128, 2, W] -> psum acc [33, W] f32 (num^T ; count).
  - The 512 k's per core stream in TWO k-parts (A: 320, B: 192).  Part
    A's entire epilogue hides under part B's DMA stream; only part B's
    epilogue is exposed, and B's batches taper (16..2 position-tiles) so
    the PE drains right behind the last DMA packet.
  - Per-part epilogue in a [128, W/4] all-partition layout: count chunks
    to 32-aligned partitions, ONE cmap matmul broadcasts count across
    the 32 C partitions; mean fused with the num remap (4 tensor_tensor
    ops reading PSUM directly); masked-diff-square with a fused row-sum
    (scalar_tensor_tensor accum_out).  Part A reduces on gpsimd
    (off the PE queue, hidden mid-stream), part B on PE.
"""

import os
import sys

import numpy as np

for _p in (
    "/opt/trn_rl_repo",
    "/root/.axon_site",
    "/root/.axon_site/_ro/trn_rl_repo",
):
    if os.path.isdir(_p) and _p not in sys.path:
        sys.path.append(_p)

import concourse.mybir as mybir  # noqa: E402
import concourse.tile as tile  # noqa: E402
from concourse import bacc, bass_utils  # noqa: E402

F32 = mybir.dt.float32
BF16 = mybir.dt.bfloat16
FP8 = mybir.dt.float8e4
AOT = mybir.AluOpType

# Problem shapes (hardcoded per contract).
N, H, W, C, K = 16, 32, 32, 32, 4096
NHW = N * H * W            # 16384 positions
NCORES = 8
KS = K // NCORES           # 512 codebook entries per core
P = 128                    # partitions
S = NHW // P               # 128 position-tiles
C1 = C + 1                 # 33 = C + ones column

KA, KB_ = 320, 192         # k-part split per core (A streams first)
WA, WB = KA // 4, KB_ // 4  # per-chunk widths (80, 48)

# Position-tile batches per part.  A ends with a small batch so its
# epilogue starts promptly; B tapers hard (its drain+epilogue is the
# exposed tail of the kernel).
BATCHES_A = [32, 32, 32, 24, 8]
BATCHES_B = [32, 32, 32, 16, 8, 6, 2]
assert sum(BATCHES_A) == S and sum(BATCHES_B) == S
assert all(gb % 2 == 0 for gb in BATCHES_A + BATCHES_B)

_CACHE: dict = {}


def _build_nc():
    """Trace + compile the per-core Bass program (identical on all cores)."""
    nc = bacc.Bacc(
        "TRN2",
        target_bir_lowering=False,
        debug=False,
        enable_asserts=False,
        num_devices=NCORES,
    )

    # code_a[p, s*KA + k] = code[s*P + p, base + k],  k in [0, KA)
    code_a_d = nc.dram_tensor("code_a", [P, S * KA], FP8, kind="ExternalInput").ap()
    # code_b[p, s*KB_ + k] = code[s*P + p, base + KA + k]
    code_b_d = nc.dram_tensor("code_b", [P, S * KB_], FP8, kind="ExternalInput").ap()
    # qo[p, (j*64 + a)*33 + c] = [quant | ones][(2a+j)*P + p, c]  (fp8)
    qo_d = nc.dram_tensor("qo", [P, S * C1], FP8, kind="ExternalInput").ap()
    # epilogue constants, packed: [cb_a | cb_b | cmap-as-f32]
    # cb_a[32j+c, x] = codebook[base + WA*j + x, c]
    # cmap[p, m] = 1 if p == 32*(m//32) else 0  (count-broadcast lhsT)
    econst_d = nc.dram_tensor(
        "econst", [P, WA + WB + P], F32, kind="ExternalInput"
    ).ap()
    loss_d = nc.dram_tensor("loss", [1, 4], F32, kind="ExternalOutput").ap()

    with tile.TileContext(nc) as tc:
        with (
            tc.tile_pool(name="consts", bufs=1) as consts,
            tc.tile_pool(name="codep", bufs=8) as codep,
            tc.tile_pool(name="work", bufs=1) as work,
            tc.tile_pool(name="acc_psum", bufs=1, space="PSUM") as acc_psum,
            tc.tile_pool(name="aux_psum", bufs=1, space="PSUM") as aux_psum,
        ):
            qo_sb = consts.tile([P, S * C1], FP8)
            econst = consts.tile([P, WA + WB + P], F32)
            cb_a_sb = econst[:, 0:WA]
            cb_b_sb = econst[:, WA : WA + WB]
            cmap_sb = econst[:, WA + WB :]
            # count chunks land on 32-aligned partitions; other rows must be
            # zero (they're contracted by the broadcast matmul).
            cntq_a = consts.tile([P, WA], F32)
            cntq_b = consts.tile([P, WB], F32)
            nc.vector.memset(cntq_a, 0.0)
            nc.vector.memset(cntq_b, 0.0)
            ones128 = consts.tile([P, 1], F32)
            nc.vector.memset(ones128, 1.0)
            fin = work.tile([1, 4], F32)

            # qo in 2 transfers (one j-block per ring); PE has enough
            # slack that matmul 0 can wait for the full qo.
            qh = (S * C1) // 2
            nc.sync.dma_start(qo_sb[:, 0:qh], qo_d[:, 0:qh])
            nc.scalar.dma_start(qo_sb[:, qh:], qo_d[:, qh:])
            qo3 = qo_sb.rearrange("p (j a c) -> p j a c", a=S // 2, c=C1)

            ring = [nc.sync, nc.scalar]
            rr = 0  # ring round-robin over the whole stream

            def stream_part(code_d, kw, acc, batches, first, after_first=None):
                nonlocal rr
                off = 0
                n_pairs = S // 2
                for t, gb in enumerate(batches):
                    if t == 1 and after_first is not None:
                        # splice the previous part's epilogue here: its PE
                        # broadcast matmul then sits AFTER this part's first
                        # batch of DR matmuls in the in-order PE queue, so
                        # the PE never stalls waiting for the count copies.
                        after_first()
                    ctile = codep.tile([P, gb * kw], FP8, tag="code")
                    if first and t == 0:
                        # split across BOTH rings in quarters; qo slice 0
                        # rides between them so matmul 0 unblocks ASAP.
                        q = gb * kw // 4
                        nc.sync.dma_start(ctile[:, 0:q], code_d[:, 0:q])
                        nc.scalar.dma_start(
                            ctile[:, q : 2 * q], code_d[:, q : 2 * q]
                        )
                        nc.sync.dma_start(
                            ctile[:, 2 * q : 3 * q], code_d[:, 2 * q : 3 * q]
                        )
                        nc.scalar.dma_start(
                            ctile[:, 3 * q :], code_d[:, 3 * q : 4 * q]
                        )
                    else:
                        ring[rr % 2].dma_start(
                            ctile, code_d[:, off * kw : (off + gb) * kw]
                        )
                        rr += 1
                    if first and t == 2:
                        # cb/cmap only gate the epilogues; issue mid-stream.
                        nc.scalar.dma_start(econst, econst_d)
                    ct3 = ctile.rearrange("p (g k) -> p g k", k=kw)
                    for b in range(gb // 2):
                        a = off // 2 + b
                        nc.tensor.matmul(
                            acc,
                            qo3[:, :, a, 0:C1],
                            ct3[:, 2 * b : 2 * b + 2, :],
                            start=(a == 0),
                            stop=(a == n_pairs - 1),
                            perf_mode=mybir.MatmulPerfMode.DoubleRow,
                        )
                    off += gb

            def epilogue(acc, w, cntq, cb_sb_p, tag, ssq_col, nv_col, pe_fin):
                # count chunk j -> cntq row 32j (DVE x2 / ACT x2, parallel)
                nc.vector.tensor_copy(cntq[0:1, :], acc[C:C1, 0:w])
                nc.vector.tensor_copy(cntq[32:33, :], acc[C:C1, w : 2 * w])
                nc.scalar.copy(cntq[64:65, :], acc[C:C1, 2 * w : 3 * w])
                nc.scalar.copy(cntq[96:97, :], acc[C:C1, 3 * w : 4 * w])
                # ONE matmul broadcasts count chunks across the C partitions
                c_ps = aux_psum.tile([P, w], F32, tag=f"c{tag}")
                nc.tensor.matmul(c_ps, cmap_sb, cntq, start=True, stop=True)

                safe = work.tile([P, w], F32, tag=f"sf{tag}")
                nc.vector.tensor_scalar_max(safe, c_ps, 0.5)
                rcp = work.tile([P, w], F32, tag=f"rc{tag}")
                nc.vector.reciprocal_approx_fast(rcp, safe)
                valid = work.tile([P, w], F32, tag=f"vl{tag}")
                nc.vector.tensor_scalar(valid, c_ps, 0.5, None, AOT.is_gt)
                # grand total of valid (= 32 * n_valid) on gpsimd, concurrent
                # with the fp32 (1-port) DVE chain
                nc.gpsimd.reduce_sum(
                    fin[:, nv_col : nv_col + 1],
                    valid,
                    axis=mybir.AxisListType.XYZWC,
                )
                # mean fused with the num remap: PSUM chunk * rcp row-block
                mean = work.tile([P, w], F32, tag=f"mn{tag}")
                for j in range(4):
                    nc.vector.tensor_mul(
                        mean[32 * j : 32 * (j + 1), :],
                        acc[0:C, w * j : w * (j + 1)],
                        rcp[32 * j : 32 * (j + 1), :],
                    )
                # e = cb - mean  (fused: (mean * -1) + cb)
                e = work.tile([P, w], F32, tag=f"e{tag}")
                nc.vector.scalar_tensor_tensor(
                    e, mean, -1.0, cb_sb_p, AOT.mult, AOT.add
                )
                dm = work.tile([P, w], F32, tag=f"dm{tag}")
                nc.vector.tensor_mul(dm, e, valid)
                dsq = work.tile([P, w], F32, tag=f"dq{tag}")
                s0 = work.tile([P, 1], F32, tag=f"s0{tag}")
                nc.vector.scalar_tensor_tensor(
                    dsq, dm, 1.0, dm, AOT.mult, AOT.mult, accum_out=s0
                )
                if pe_fin:
                    fin_ps = aux_psum.tile([1, 1], F32, tag=f"f{tag}")
                    nc.tensor.matmul(fin_ps, ones128, s0, start=True, stop=True)
                    nc.vector.tensor_copy(fin[:, ssq_col : ssq_col + 1], fin_ps)
                else:
                    # gpsimd partition reduce: keeps part A's tail off the PE
                    # queue (which is busy with part B's stream matmuls)
                    nc.gpsimd.reduce_sum(
                        fin[:, ssq_col : ssq_col + 1],
                        s0,
                        axis=mybir.AxisListType.C,
                    )

            acc_a = acc_psum.tile([C1, KA], F32, tag="acca")
            acc_b = acc_psum.tile([C1, KB_], F32, tag="accb")

            stream_part(code_a_d, KA, acc_a, BATCHES_A, first=True)
            stream_part(
                code_b_d, KB_, acc_b, BATCHES_B, first=False,
                after_first=lambda: epilogue(
                    acc_a, WA, cntq_a, cb_a_sb, "a", 0, 1, pe_fin=False
                ),
            )
            epilogue(acc_b, WB, cntq_b, cb_b_sb, "b", 2, 3, pe_fin=True)

            nc.sync.dma_start(loss_d, fin)

    nc.compile()
    return nc


def _get_nc():
    if "nc" not in _CACHE:
        _CACHE["nc"] = _build_nc()
    return _CACHE["nc"]


def _make_in_maps(quantized, code, codebook):
    np_fp8 = mybir.dt.np(FP8)

    q2 = np.asarray(quantized, dtype=np.float32).reshape(NHW, C)
    code2 = np.asarray(code, dtype=np.float32).reshape(NHW, K)
    cb = np.asarray(codebook, dtype=np.float32)

    qo = np.zeros((NHW, C1), np.float32)
    qo[:, 0:C] = q2
    qo[:, C] = 1.0
    # qo_kc[p, (j*64 + a)*33 + c] = qo[(2a+j)*128 + p, c]  (j-major blocks
    # so the DoubleRow pair stride is 64*33 = 2112, a multiple of 16)
    qo_kc = np.ascontiguousarray(
        qo.reshape(S // 2, 2, P, C1).transpose(2, 1, 0, 3)
    ).reshape(P, S * C1).astype(np_fp8)

    # cmap[p, m] = 1 if p == 32*(m//32) else 0
    cmap = np.zeros((P, P), np.float32)
    for j in range(4):
        cmap[32 * j, 32 * j : 32 * (j + 1)] = 1.0

    code8 = code2.astype(np_fp8)  # 0/1 values: exact
    in_maps = []
    for j in range(NCORES):
        base = j * KS
        # [NHW, kw] -> [S, P, kw] -> [P, S, kw] -> [128, S*kw]
        code_a = np.ascontiguousarray(
            code8[:, base : base + KA].reshape(S, P, KA).swapaxes(0, 1)
        ).reshape(P, S * KA)
        code_b = np.ascontiguousarray(
            code8[:, base + KA : base + KS].reshape(S, P, KB_).swapaxes(0, 1)
        ).reshape(P, S * KB_)
        # cb_x[32u+c, x] = cb[base(+KA) + W*u + x, c]
        cb_a = np.ascontiguousarray(
            cb[base : base + KA].reshape(4, WA, C).transpose(0, 2, 1)
        ).reshape(P, WA)
        cb_b = np.ascontiguousarray(
            cb[base + KA : base + KS].reshape(4, WB, C).transpose(0, 2, 1)
        ).reshape(P, WB)
        econst = np.concatenate([cb_a, cb_b, cmap], axis=1)
        in_maps.append(
            {
                "code_a": code_a,
                "code_b": code_b,
                "qo": qo_kc,
                "econst": np.ascontiguousarray(econst),
            }
        )
    return in_maps


def run(quantized, code, codebook, trace=False, **spmd_kwargs):
    """Run the SPMD kernel; returns (loss_scalar, BassKernelResults)."""
    nc = _get_nc()
    in_maps = _make_in_maps(quantized, code, codebook)
    res = bass_utils.run_bass_kernel_spmd(
        nc, in_maps, core_ids=list(range(NCORES)), trace=trace, **spmd_kwargs
    )
    parts = np.stack(
        [np.asarray(res.results[j]["loss"]).reshape(4) for j in range(NCORES)]
    )
    tot = parts.sum(axis=0, dtype=np.float32)
    ssq = tot[0] + tot[2]
    nv32 = tot[1] + tot[3]  # 32 * n_valid
    loss = np.float32(ssq / max(nv32, np.float32(C)))
    return np.asarray(loss, dtype=np.float32).reshape(()), res


def kernel(quantized, code, codebook):
    loss, _ = run(quantized, code, codebook)
    return loss


# revision 24
# speedup vs baseline: 1.0039x; 1.0039x over previous
"""Trainium2 Bass kernel for nn_MeanAligning (VQ codebook mean-aligning loss).

Math (see reference):
    count[k] = sum_nhw code[nhw, k]
    num[k,c] = sum_nhw code[nhw, k] * quantized[nhw, c]
    mean     = num / count (count==0 -> mean 0)
    loss     = sum_{k: count>0} ||codebook[k] - mean[k]||^2 / (n_valid * C)

Sharding: K-parallel over the 4096 codebook entries — each of the 8 cores
gets a contiguous 512-column slice of `code` and ALL positions, so each
core owns the *complete* count/num for its K-shard. Only a tiny [1,4]
partial crosses cores at the end (summed on host as the unshard step).

Device pipeline per core:
  - `code` is staged host-side as fp8e4 (one-hot 0/1 values are exact in
    fp8e4 — a lossless relayout, 4x less HBM traffic than f32) and
    `quant|ones` as fp8e4 in j-major blocks so the DoubleRow pair stride
    (64*33 = 2112) is 16-aligned with no padding.
  - PSUM-accumulated DoubleRow matmuls: lhsT = [quant|ones] [128, 2, 33],
    rhs = code [128, 2, W] -> psum acc [33, W] f32 (num^T ; count).
  - The 512 k's per core stream in TWO k-parts (A: 320, B: 192).  Part
    A's entire epilogue hides under part B's DMA stream; only part B's
    epilogue is exposed, and B's batches taper (16..2 position-tiles) so
    the PE drains right behind the last DMA packet.
  - Per-part epilogue in a [128, W/4] all-partition layout: count chunks
    to 32-aligned partitions, ONE cmap matmul broadcasts count across
    the 32 C partitions; mean fused with the num remap (4 tensor_tensor
    ops reading PSUM directly); masked-diff-square with a fused row-sum
    (scalar_tensor_tensor accum_out).  Part A reduces on gpsimd
    (off the PE queue, hidden mid-stream), part B on PE.
"""

import os
import sys

import numpy as np

for _p in (
    "/opt/trn_rl_repo",
    "/root/.axon_site",
    "/root/.axon_site/_ro/trn_rl_repo",
):
    if os.path.isdir(_p) and _p not in sys.path:
        sys.path.append(_p)

import concourse.mybir as mybir  # noqa: E402
import concourse.tile as tile  # noqa: E402
from concourse import bacc, bass_utils  # noqa: E402

F32 = mybir.dt.float32
BF16 = mybir.dt.bfloat16
FP8 = mybir.dt.float8e4
AOT = mybir.AluOpType

# Problem shapes (hardcoded per contract).
N, H, W, C, K = 16, 32, 32, 32, 4096
NHW = N * H * W            # 16384 positions
NCORES = 8
KS = K // NCORES           # 512 codebook entries per core
P = 128                    # partitions
S = NHW // P               # 128 position-tiles
C1 = C + 1                 # 33 = C + ones column

KA, KB_ = 320, 192         # k-part split per core (A streams first)
WA, WB = KA // 4, KB_ // 4  # per-chunk widths (80, 48)

# Position-tile batches per part.  A ends with a small batch so its
# epilogue starts promptly; B tapers hard (its drain+epilogue is the
# exposed tail of the kernel).
# Uniform 5120-byte/partition DMA slots: A batches are exactly 16 tiles
# (16*320B); B batches fit the same slot (26*192B = 4992).  The 5-deep
# code pool provides the backpressure that keeps part B's transfers from
# starting (and stealing HBM round-robin bandwidth) before part A's tail
# has landed.
BATCHES_A = [16, 16, 16, 16, 16, 16, 16, 16]
BATCHES_B = [26, 26, 26, 24, 14, 8, 4]
assert sum(BATCHES_A) == S and sum(BATCHES_B) == S
assert all(gb % 2 == 0 for gb in BATCHES_A + BATCHES_B)

_CACHE: dict = {}


def _build_nc():
    """Trace + compile the per-core Bass program (identical on all cores)."""
    nc = bacc.Bacc(
        "TRN2",
        target_bir_lowering=False,
        debug=False,
        enable_asserts=False,
        num_devices=NCORES,
    )

    # code_a[p, s*KA + k] = code[s*P + p, base + k],  k in [0, KA)
    code_a_d = nc.dram_tensor("code_a", [P, S * KA], FP8, kind="ExternalInput").ap()
    # code_b[p, s*KB_ + k] = code[s*P + p, base + KA + k]
    code_b_d = nc.dram_tensor("code_b", [P, S * KB_], FP8, kind="ExternalInput").ap()
    # qo[p, (j*64 + a)*33 + c] = [quant | ones][(2a+j)*P + p, c]  (fp8)
    qo_d = nc.dram_tensor("qo", [P, S * C1], FP8, kind="ExternalInput").ap()
    # epilogue constants, packed: [cb_a | cb_b | cmap-as-f32]
    # cb_a[32j+c, x] = codebook[base + WA*j + x, c]
    # cmap[p, m] = 1 if p == 32*(m//32) else 0  (count-broadcast lhsT)
    econst_d = nc.dram_tensor(
        "econst", [P, WA + WB + P], F32, kind="ExternalInput"
    ).ap()
    loss_d = nc.dram_tensor("loss", [1, 4], F32, kind="ExternalOutput").ap()

    with tile.TileContext(nc) as tc:
        with (
            tc.tile_pool(name="consts", bufs=1) as consts,
            tc.tile_pool(name="codep", bufs=5) as codep,
            tc.tile_pool(name="work", bufs=1) as work,
            tc.tile_pool(name="acc_psum", bufs=1, space="PSUM") as acc_psum,
            tc.tile_pool(name="aux_psum", bufs=1, space="PSUM") as aux_psum,
        ):
            qo_sb = consts.tile([P, S * C1], FP8)
            econst = consts.tile([P, WA + WB + P], F32)
            cb_a_sb = econst[:, 0:WA]
            cb_b_sb = econst[:, WA : WA + WB]
            cmap_sb = econst[:, WA + WB :]
            # count chunks land on 32-aligned partitions; other rows must be
            # zero (they're contracted by the broadcast matmul).
            cntq_a = consts.tile([P, WA], F32)
            cntq_b = consts.tile([P, WB], F32)
            nc.vector.memset(cntq_a, 0.0)
            nc.vector.memset(cntq_b, 0.0)
            ones128 = consts.tile([P, 1], F32)
            nc.vector.memset(ones128, 1.0)
            fin = work.tile([1, 4], F32)

            # qo in 2 transfers (one j-block per ring); PE has enough
            # slack that matmul 0 can wait for the full qo.
            qh = (S * C1) // 2
            nc.sync.dma_start(qo_sb[:, 0:qh], qo_d[:, 0:qh])
            nc.scalar.dma_start(qo_sb[:, qh:], qo_d[:, qh:])
            qo3 = qo_sb.rearrange("p (j a c) -> p j a c", a=S // 2, c=C1)

            ring = [nc.sync, nc.scalar]
            rr = 0  # ring round-robin over the whole stream

            def stream_part(code_d, kw, acc, batches, first, after_first=None):
                nonlocal rr
                off = 0
                n_pairs = S // 2
                slot = 16 * KA  # uniform pool slot (5120 B/partition)
                for t, gb in enumerate(batches):
                    if t == 1 and after_first is not None:
                        # splice the previous part's epilogue here: its PE
                        # broadcast matmul then sits AFTER this part's first
                        # batch of DR matmuls in the in-order PE queue, so
                        # the PE never stalls waiting for the count copies.
                        after_first()
                    cslot = codep.tile([P, slot], FP8, tag="code")
                    ctile = cslot[:, 0 : gb * kw]
                    if first and t == 0:
                        # split across BOTH rings in quarters; qo slice 0
                        # rides between them so matmul 0 unblocks ASAP.
                        q = gb * kw // 4
                        nc.sync.dma_start(ctile[:, 0:q], code_d[:, 0:q])
                        nc.scalar.dma_start(
                            ctile[:, q : 2 * q], code_d[:, q : 2 * q]
                        )
                        nc.sync.dma_start(
                            ctile[:, 2 * q : 3 * q], code_d[:, 2 * q : 3 * q]
                        )
                        nc.scalar.dma_start(
                            ctile[:, 3 * q :], code_d[:, 3 * q : 4 * q]
                        )
                    else:
                        ring[rr % 2].dma_start(
                            ctile, code_d[:, off * kw : (off + gb) * kw]
                        )
                        rr += 1
                    if first and t == 2:
                        # cb/cmap only gate the epilogues; issue mid-stream.
                        nc.scalar.dma_start(econst, econst_d)
                    ct3 = ctile.rearrange("p (g k) -> p g k", k=kw)
                    for b in range(gb // 2):
                        a = off // 2 + b
                        nc.tensor.matmul(
                            acc,
                            qo3[:, :, a, 0:C1],
                            ct3[:, 2 * b : 2 * b + 2, :],
                            start=(a == 0),
                            stop=(a == n_pairs - 1),
                            perf_mode=mybir.MatmulPerfMode.DoubleRow,
                        )
                    off += gb

            def epilogue(acc, w, cntq, cb_sb_p, tag, ssq_col, nv_col, pe_fin):
                # count chunk j -> cntq row 32j (DVE x2 / ACT x2, parallel)
                nc.vector.tensor_copy(cntq[0:1, :], acc[C:C1, 0:w])
                nc.vector.tensor_copy(cntq[32:33, :], acc[C:C1, w : 2 * w])
                nc.scalar.copy(cntq[64:65, :], acc[C:C1, 2 * w : 3 * w])
                nc.scalar.copy(cntq[96:97, :], acc[C:C1, 3 * w : 4 * w])
                # ONE matmul broadcasts count chunks across the C partitions
                c_ps = aux_psum.tile([P, w], F32, tag=f"c{tag}")
                nc.tensor.matmul(c_ps, cmap_sb, cntq, start=True, stop=True)

                safe = work.tile([P, w], F32, tag=f"sf{tag}")
                nc.vector.tensor_scalar_max(safe, c_ps, 0.5)
                rcp = work.tile([P, w], F32, tag=f"rc{tag}")
                nc.vector.reciprocal_approx_fast(rcp, safe)
                valid = work.tile([P, w], F32, tag=f"vl{tag}")
                nc.vector.tensor_scalar(valid, c_ps, 0.5, None, AOT.is_gt)
                # grand total of valid (= 32 * n_valid) on gpsimd, concurrent
                # with the fp32 (1-port) DVE chain
                nc.gpsimd.reduce_sum(
                    fin[:, nv_col : nv_col + 1],
                    valid,
                    axis=mybir.AxisListType.XYZWC,
                )
                # mean fused with the num remap: PSUM chunk * rcp row-block
                mean = work.tile([P, w], F32, tag=f"mn{tag}")
                for j in range(4):
                    nc.vector.tensor_mul(
                        mean[32 * j : 32 * (j + 1), :],
                        acc[0:C, w * j : w * (j + 1)],
                        rcp[32 * j : 32 * (j + 1), :],
                    )
                # e = cb - mean  (fused: (mean * -1) + cb)
                e = work.tile([P, w], F32, tag=f"e{tag}")
                nc.vector.scalar_tensor_tensor(
                    e, mean, -1.0, cb_sb_p, AOT.mult, AOT.add
                )
                dm = work.tile([P, w], F32, tag=f"dm{tag}")
                nc.vector.tensor_mul(dm, e, valid)
                dsq = work.tile([P, w], F32, tag=f"dq{tag}")
                s0 = work.tile([P, 1], F32, tag=f"s0{tag}")
                nc.vector.scalar_tensor_tensor(
                    dsq, dm, 1.0, dm, AOT.mult, AOT.mult, accum_out=s0
                )
                if pe_fin:
                    fin_ps = aux_psum.tile([1, 1], F32, tag=f"f{tag}")
                    nc.tensor.matmul(fin_ps, ones128, s0, start=True, stop=True)
                    nc.vector.tensor_copy(fin[:, ssq_col : ssq_col + 1], fin_ps)
                else:
                    # gpsimd partition reduce: keeps part A's tail off the PE
                    # queue (which is busy with part B's stream matmuls)
                    nc.gpsimd.reduce_sum(
                        fin[:, ssq_col : ssq_col + 1],
                        s0,
                        axis=mybir.AxisListType.C,
                    )

            acc_a = acc_psum.tile([C1, KA], F32, tag="acca")
            acc_b = acc_psum.tile([C1, KB_], F32, tag="accb")

            stream_part(code_a_d, KA, acc_a, BATCHES_A, first=True)
            stream_part(
                code_b_d, KB_, acc_b, BATCHES_B, first=False,
                after_first=lambda: epilogue(
                    acc_a, WA, cntq_a, cb_a_sb, "a", 0, 1, pe_fin=False
                ),
            )
            epilogue(acc_b, WB, cntq_b, cb_b_sb, "b", 2, 3, pe_fin=True)

            nc.sync.dma_start(loss_d, fin)

    nc.compile()
    return nc


def _get_nc():
    if "nc" not in _CACHE:
        _CACHE["nc"] = _build_nc()
    return _CACHE["nc"]


def _make_in_maps(quantized, code, codebook):
    np_fp8 = mybir.dt.np(FP8)

    q2 = np.asarray(quantized, dtype=np.float32).reshape(NHW, C)
    code2 = np.asarray(code, dtype=np.float32).reshape(NHW, K)
    cb = np.asarray(codebook, dtype=np.float32)

    qo = np.zeros((NHW, C1), np.float32)
    qo[:, 0:C] = q2
    qo[:, C] = 1.0
    # qo_kc[p, (j*64 + a)*33 + c] = qo[(2a+j)*128 + p, c]  (j-major blocks
    # so the DoubleRow pair stride is 64*33 = 2112, a multiple of 16)
    qo_kc = np.ascontiguousarray(
        qo.reshape(S // 2, 2, P, C1).transpose(2, 1, 0, 3)
    ).reshape(P, S * C1).astype(np_fp8)

    # cmap[p, m] = 1 if p == 32*(m//32) else 0
    cmap = np.zeros((P, P), np.float32)
    for j in range(4):
        cmap[32 * j, 32 * j : 32 * (j + 1)] = 1.0

    code8 = code2.astype(np_fp8)  # 0/1 values: exact
    in_maps = []
    for j in range(NCORES):
        base = j * KS
        # [NHW, kw] -> [S, P, kw] -> [P, S, kw] -> [128, S*kw]
        code_a = np.ascontiguousarray(
            code8[:, base : base + KA].reshape(S, P, KA).swapaxes(0, 1)
        ).reshape(P, S * KA)
        code_b = np.ascontiguousarray(
            code8[:, base + KA : base + KS].reshape(S, P, KB_).swapaxes(0, 1)
        ).reshape(P, S * KB_)
        # cb_x[32u+c, x] = cb[base(+KA) + W*u + x, c]
        cb_a = np.ascontiguousarray(
            cb[base : base + KA].reshape(4, WA, C).transpose(0, 2, 1)
        ).reshape(P, WA)
        cb_b = np.ascontiguousarray(
            cb[base + KA : base + KS].reshape(4, WB, C).transpose(0, 2, 1)
        ).reshape(P, WB)
        econst = np.concatenate([cb_a, cb_b, cmap], axis=1)
        in_maps.append(
            {
                "code_a": code_a,
                "code_b": code_b,
                "qo": qo_kc,
                "econst": np.ascontiguousarray(econst),
            }
        )
    return in_maps


def run(quantized, code, codebook, trace=False, **spmd_kwargs):
    """Run the SPMD kernel; returns (loss_scalar, BassKernelResults)."""
    nc = _get_nc()
    in_maps = _make_in_maps(quantized, code, codebook)
    res = bass_utils.run_bass_kernel_spmd(
        nc, in_maps, core_ids=list(range(NCORES)), trace=trace, **spmd_kwargs
    )
    parts = np.stack(
        [np.asarray(res.results[j]["loss"]).reshape(4) for j in range(NCORES)]
    )
    tot = parts.sum(axis=0, dtype=np.float32)
    ssq = tot[0] + tot[2]
    nv32 = tot[1] + tot[3]  # 32 * n_valid
    loss = np.float32(ssq / max(nv32, np.float32(C)))
    return np.asarray(loss, dtype=np.float32).reshape(()), res


def kernel(quantized, code, codebook):
    loss, _ = run(quantized, code, codebook)
    return loss
